# revision 10
# baseline (speedup 1.0000x reference)
"""GCN layer (symmetric-normalized message passing + skip) on 8 Trainium2
NeuronCores via Bass/Tile.

    deg = bincount(src); dis = (deg>0) * rsqrt(max(deg,1))
    out = segsum_dst( dis_src*dis_dst * feats[src] ) @ Wm.T + bm
          + feats @ Ws.T + bs

v3 design (vs v2's dynamic one-hot): STATIC scatter masks. Nodes are sorted
globally by in-degree and dealt round-robin to the 8 cores, so every core
sees the same degree profile. Each 512-dst superblock assigns every dst a
fixed, even K = max in-degree in the block rounded up; edge j of dst d sits
at slot d*K+j. The scatter matrix for each 128-slot tile is then a STATIC
0/1 band that depends only on (K, 128t mod K) -- 19 distinct [128,1024] f16
mask buffers loaded once, with the matmul rhs a 512-wide column slice.
This removes the per-tile DVE one-hot build (v2's #1 compute load) and the
per-edge ACT scaling entirely: the only per-tile vector work is one
[128,128] tensor_scalar multiplying the gathered messages by the per-slot
weight w = dis[src]*dis[dst] (computed on device from integer degrees; pad
slots have w=0, which also nullifies garbage gather rows).

Gather: superblock sb uses a per-(core, sb%4) compacted source table
(<32768 rows, so one int16-indexed dma_gather covers a whole slot range);
each superblock's rows split into 4 chunks, one per SWDGE queue, which run
concurrently on separate Q7 core pairs. Trailing pad slots per chunk get
idx=-1, which the ucode trims for free.

Host prep supplies integer partitioning metadata only (permutations, table
row ids, per-slot degrees); all float math runs on device.
"""

import numpy as np

P = 128
D = 128
NCORES = 8
N = 100000
E = 640000
SBW = 512                   # superblock width (dst nodes)
NSB = 25                    # superblocks per core (25*512 = 12800 >= 12500)
NLOC = N // NCORES          # 12500
NLOC_PAD = NSB * SBW        # 12800
NPHASE = 4                  # sb % 4 -> which compacted source table
MAXIDX = 32768              # int16 gather index limit


# ---------------------------------------------------------------- host prep

def _prep(feats, src, dst, wm, bm, ws, bs):
    n, d = feats.shape
    assert n == N and d == D
    src = np.asarray(src).astype(np.int64)
    dst = np.asarray(dst).astype(np.int64)
    feats = np.asarray(feats, dtype=np.float32)

    deg = np.bincount(src, minlength=n)            # out-degree -> dis
    keep = deg[dst] > 0                            # dis[dst]=0 edges are 0
    src_k, dst_k = src[keep], dst[keep]
    deg_in = np.bincount(dst_k, minlength=n)

    order = np.argsort(-deg_in, kind="stable")     # rank -> node
    rank_of = np.empty(n, np.int64)
    rank_of[order] = np.arange(n)
    owner = rank_of % NCORES
    lrank = rank_of // NCORES

    Ks = []
    for sb in range(NSB):
        band = deg_in[order[sb * SBW * NCORES:(sb + 1) * SBW * NCORES]]
        K = max(2, int(band.max()))
        K += K % 2                                 # even K: few mask bufs
        Ks.append(K)
    Ks = np.array(Ks)
    tiles_sb = (SBW * Ks // P).astype(np.int64)    # = 4*K
    slot_base = np.concatenate([[0], np.cumsum(SBW * Ks)]).astype(np.int64)
    TOT = int(slot_base[-1])
    TILES = TOT // P

    # static mask buffers, one per distinct (K, c); band at col 512+(c+p)//K
    mask_key = {}
    bufs = []
    for sb in range(NSB):
        K = int(Ks[sb])
        for t in range(int(tiles_sb[sb])):
            c = (P * t) % K
            if (K, c) not in mask_key:
                buf = np.zeros((P, 1024), np.float16)
                pp = np.arange(P)
                buf[pp, 512 + (c + pp) // K] = 1.0
                mask_key[(K, c)] = len(bufs)
                bufs.append(buf)
    NMASK = len(bufs)
    masks = np.zeros((P, NMASK * 1024), np.float16)
    for mi, buf in enumerate(bufs):
        masks[:, mi * 1024:(mi + 1) * 1024] = buf

    # per-tile mask slice (buffer id, column start) -- same for all cores
    tile_mask = []                                 # [(mi, col_start)] * TILES
    for sb in range(NSB):
        K = int(Ks[sb])
        for t in range(int(tiles_sb[sb])):
            c = (P * t) % K
            d0 = (P * t) // K
            mi = mask_key[(K, c)]
            tile_mask.append((mi, mi * 1024 + 512 - d0))

    # gather chunking: ops of <=7 tiles (896 rows; 57 descs/engine fits the
    # SWDGE ring), round-robin across the 4 queues
    TPO = 7
    chunks = []                                    # (sb, tile_lo, tile_hi, q)
    qctr = 0
    for sb in range(NSB):
        tsb = int(tiles_sb[sb])
        t0 = int(slot_base[sb]) // P
        for t in range(0, tsb, TPO):
            lo, hi = t0 + t, t0 + min(tsb, t + TPO)
            chunks.append((sb, lo, hi, qctr % 4))
            qctr += 1
    # gidx column base per chunk: rows/16 columns each, laid sequentially
    chunk_cols = []
    cb = 0
    for (_, lo, hi, _) in chunks:
        nc_ = (hi - lo) * P // 16
        chunk_cols.append((cb, nc_))
        cb += nc_
    GIDXC = cb                                     # == TOT // 16

    # per-core tables and index/degree metadata
    per_core = []
    for k in range(NCORES):
        m = owner[dst_k] == k
        s_e, d_e = src_k[m], dst_k[m]
        lr = lrank[d_e]
        o2 = np.argsort(lr, kind="stable")
        s_e, d_e, lr = s_e[o2], d_e[o2], lr[o2]
        sb_e, dloc = lr // SBW, lr % SBW
        j = np.arange(len(lr)) - np.searchsorted(lr, lr)
        slot = slot_base[sb_e] + dloc * Ks[sb_e] + j

        phase_e = sb_e % NPHASE
        tabs = []
        gidx_flat = np.zeros(TOT, np.int64)
        for ph in range(NPHASE):
            mm2 = phase_e == ph
            uniq = np.unique(s_e[mm2])
            assert len(uniq) < MAXIDX, len(uniq)
            r = np.zeros(n, np.int64)
            r[uniq] = np.arange(len(uniq))
            tabs.append(feats[uniq].astype(np.float16))
            gidx_flat[slot[mm2]] = r[s_e[mm2]]

        filled = np.zeros(TOT, bool)
        filled[slot] = True
        degS = np.zeros(TOT, np.int16)
        degD = np.zeros(TOT, np.int16)
        degS[slot] = deg[s_e].astype(np.int16)
        degD[slot] = deg[d_e].astype(np.int16)

        # trailing pad slots per chunk -> idx -1 (ucode trims them)
        for (sb, lo, hi, _), (cb0, ncol) in zip(chunks, chunk_cols):
            s0, s1 = lo * P, hi * P
            f = filled[s0:s1]
            nz = np.flatnonzero(f)
            tail = (nz[-1] + 1) if len(nz) else 0
            gidx_flat[s0 + tail:s1] = -1

        # wrap-16 + replicate-8 packing per chunk
        gidx = np.zeros((P, GIDXC), np.int16)
        for (sb, lo, hi, _), (cb0, ncol) in zip(chunks, chunk_cols):
            fl = gidx_flat[lo * P:hi * P].astype(np.int16)
            a = fl.reshape(ncol, 16).T              # [16, ncol]
            gidx[:, cb0:cb0 + ncol] = np.tile(a, (8, 1))

        gdegS = np.ascontiguousarray(degS.reshape(TILES, P).T)
        gdegD = np.ascontiguousarray(degD.reshape(TILES, P).T)

        ft = np.zeros((P, NLOC_PAD), np.float16)
        rr = np.arange(NLOC)
        ft[:, :NLOC] = feats[order[rr * NCORES + k]].T.astype(np.float16)

        per_core.append((tabs, gidx, gdegS, gdegD, ft))

    # uniform table shape across cores/phases
    TABR = max(t.shape[0] for (tabs, *_r) in per_core for t in tabs)
    TABR = (TABR + 127) // 128 * 128

    wmT = np.ascontiguousarray(np.asarray(wm, np.float32).T).astype(np.float16)
    wsT = np.ascontiguousarray(np.asarray(ws, np.float32).T).astype(np.float16)
    bm = np.asarray(bm, np.float32).reshape(1, D)
    bs = np.asarray(bs, np.float32).reshape(1, D)

    in_maps = []
    for k in range(NCORES):
        tabs, gidx, gdegS, gdegD, ft = per_core[k]
        im = {"gidx": gidx, "gdegS": gdegS, "gdegD": gdegD,
              "featsT": ft, "masks": masks,
              "wmT": wmT, "wsT": wsT, "bm": bm, "bs": bs}
        for ph in range(NPHASE):
            tt = np.zeros((TABR, D), np.float16)
            tt[:tabs[ph].shape[0]] = tabs[ph]
            im[f"tab{ph}"] = tt
        in_maps.append(im)

    geom = {
        "Ks": Ks.tolist(), "tiles_sb": tiles_sb.tolist(),
        "slot_base": slot_base.tolist(), "TOT": TOT, "TILES": TILES,
        "NMASK": NMASK, "tile_mask": tile_mask, "chunks": chunks,
        "chunk_cols": chunk_cols, "GIDXC": GIDXC, "TABR": TABR,
    }
    return in_maps, geom, order


# ------------------------------------------------------------- device kernel

def device_kernel(tc, outs, ins, geom, cfg):
    import concourse.mybir as mybir

    nc = tc.nc
    f32 = mybir.dt.float32
    f16 = mybir.dt.float16
    i16 = mybir.dt.int16
    Op = mybir.AluOpType
    Act = mybir.ActivationFunctionType

    (out_d,) = outs
    (gidx_d, gdegS_d, gdegD_d, featsT_d, masks_d,
     wmT_d, wsT_d, bm_d, bs_d, tab_ds) = ins

    TILES = geom["TILES"]
    NMASK = geom["NMASK"]
    GIDXC = geom["GIDXC"]
    tiles_sb = geom["tiles_sb"]
    slot_base = geom["slot_base"]
    tile_mask = geom["tile_mask"]
    chunks = geom["chunks"]
    chunk_cols = geom["chunk_cols"]

    NBUF = 3
    bufmax = [max(tiles_sb[sb] for sb in range(i, NSB, NBUF))
              for i in range(NBUF)]

    abl = cfg.get("ABL", ())

    with (
        tc.tile_pool(name="sbuf", bufs=1) as sb,
        tc.tile_pool(name="srst", bufs=2) as srst,
        tc.tile_pool(name="sstg", bufs=4) as sstg,
        tc.tile_pool(name="psag", bufs=2, space="PSUM") as psag,
        tc.tile_pool(name="pslin", bufs=4, space="PSUM") as pslin,
    ):
        # ---------------- setup ----------------
        # gidx first: sb0 gathers depend only on it (+ tables already in
        # HBM), so they issue under the remaining input loads + w compute.
        gidx = sb.tile([P, GIDXC], i16)
        nc.sync.dma_start(out=gidx[:], in_=gidx_d[:])

        msgs0 = sb.tile([P, bufmax[0] * P], f16, tag="msgs0")
        msgs1 = sb.tile([P, bufmax[1] * P], f16, tag="msgs1")
        msgs2 = sb.tile([P, bufmax[2] * P], f16, tag="msgs2")
        msgs_bufs = [msgs0, msgs1, msgs2]

        sb_chunks = {}
        for (sbi, lo, hi, q), (cb0, ncol) in zip(chunks, chunk_cols):
            sb_chunks.setdefault(sbi, []).append((lo, hi, q, cb0, ncol))

        def issue_gathers(sbi):
            msgs = msgs_bufs[sbi % NBUF]
            if "gather" in abl:
                return
            t0 = slot_base[sbi] // P
            for (lo, hi, q, cb0, ncol) in sb_chunks[sbi]:
                nrow = (hi - lo) * P
                nc.gpsimd.dma_gather(
                    msgs[:, (lo - t0) * P:(hi - t0) * P]
                    .rearrange("p (t e) -> p t e", e=D),
                    tab_ds[sbi % NPHASE][:, :],
                    gidx[:, cb0:cb0 + ncol],
                    nrow, nrow, D, queue_num=q)

        issue_gathers(0)

        maskt = sb.tile([P, NMASK * 1024], f16)
        nc.sync.dma_start(out=maskt[:], in_=masks_d[:])
        wmT = sb.tile([P, D], f16)
        nc.sync.dma_start(out=wmT[:], in_=wmT_d[:])
        wsT = sb.tile([P, D], f16)
        nc.sync.dma_start(out=wsT[:], in_=wsT_d[:])
        featsT = sb.tile([P, NLOC_PAD], f16)
        nc.sync.dma_start(out=featsT[:], in_=featsT_d[:])

        # bias16 = (bm + bs) as f16 row
        bmt = sb.tile([1, D], f32)
        nc.sync.dma_start(out=bmt[:], in_=bm_d[:])
        bst = sb.tile([1, D], f32)
        nc.sync.dma_start(out=bst[:], in_=bs_d[:])
        nc.vector.tensor_tensor(out=bmt[:], in0=bmt[:], in1=bst[:], op=Op.add)
        bias16 = sb.tile([1, D], f16)
        nc.vector.tensor_copy(out=bias16[:], in_=bmt[:])
        ones1 = sb.tile([1, P], f16)
        nc.vector.memset(ones1[:], 1.0)

        # wE[p, t] = dis(degS)*dis(degD), dis(x) = (x>0)*rsqrt(max(x,1))
        def dis_of(deg_d):
            di = sb.tile([P, TILES], i16, tag=f"digi{id(deg_d)}")
            nc.sync.dma_start(out=di[:], in_=deg_d[:])
            df = sb.tile([P, TILES], f32, tag="digf")
            nc.vector.tensor_copy(out=df[:], in_=di[:])
            msk = sb.tile([P, TILES], f32, tag="dmsk")
            nc.vector.tensor_scalar(out=msk[:], in0=df[:], scalar1=0.0,
                                    scalar2=None, op0=Op.is_gt)
            nc.vector.tensor_scalar(out=df[:], in0=df[:], scalar1=1.0,
                                    scalar2=None, op0=Op.max)
            rc = sb.tile([P, TILES], f32, tag="drc")
            nc.vector.reciprocal(out=rc[:], in_=df[:])
            rt = sb.tile([P, TILES], f32, tag="drt")
            nc.scalar.activation(out=rt[:], in_=rc[:], func=Act.Sqrt)
            w = sb.tile([P, TILES], f32, tag=f"dw{id(deg_d)}")
            nc.vector.tensor_tensor(out=w[:], in0=rt[:], in1=msk[:],
                                    op=Op.mult)
            return w

        wS = dis_of(gdegS_d)
        wD = dis_of(gdegD_d)
        wE = sb.tile([P, TILES], f32)
        nc.vector.tensor_tensor(out=wE[:], in0=wS[:], in1=wD[:], op=Op.mult)

        # ---------------- main loop ----------------
        def tiles_of(sbi):
            msgs = msgs_bufs[sbi % NBUF]
            t0 = slot_base[sbi] // P
            tsb = tiles_sb[sbi]
            bankA = psag.tile([P, SBW], f32, tag="aggA", space="PSUM")
            bankB = psag.tile([P, SBW], f32, tag="aggB", space="PSUM")
            for t in range(tsb):
                T0 = t0 + t
                if "scale" not in abl:
                    nc.vector.tensor_scalar(
                        out=msgs[:, t * P:(t + 1) * P],
                        in0=msgs[:, t * P:(t + 1) * P],
                        scalar1=wE[:, T0:T0 + 1],
                        scalar2=None, op0=Op.mult)
                if "aggmm" in abl:
                    continue
                mi, cs = tile_mask[T0]
                bank = bankA if t % 2 == 0 else bankB
                nc.tensor.matmul(
                    out=bank[:], lhsT=msgs[:, t * P:(t + 1) * P],
                    rhs=maskt[:, cs:cs + SBW],
                    start=(t < 2), stop=(t >= tsb - 2))
            return bankA, bankB

        def flush_of(sbi, bankA, bankB):
            if "aggmm" in abl or "flush" in abl:
                return
            rstT = srst.tile([P, SBW], f16, tag="rstT")
            nc.scalar.copy(out=rstT[:], in_=bankA[:])
            nc.vector.tensor_tensor(out=rstT[:], in0=bankB[:],
                                    in1=rstT[:], op=Op.add)
            for b in range(4):
                pmk = pslin.tile([P, D], f32, tag="pmk", space="PSUM")
                nc.tensor.matmul(out=pmk[:],
                                 lhsT=rstT[:, b * P:(b + 1) * P],
                                 rhs=wmT[:], start=True, stop=False)
                nc.tensor.matmul(out=pmk[:],
                                 lhsT=featsT[:, (sbi * 4 + b) * P:
                                             (sbi * 4 + b + 1) * P],
                                 rhs=wsT[:], start=False, stop=False)
                nc.tensor.matmul(out=pmk[:], lhsT=ones1[:], rhs=bias16[:],
                                 start=False, stop=True)
                stage = sstg.tile([P, D], f32, tag="stage")
                nc.scalar.copy(out=stage[:], in_=pmk[:])
                nc.sync.dma_start(
                    out=out_d[(sbi * 4 + b) * P:(sbi * 4 + b + 1) * P, :],
                    in_=stage[:])

        def body():
            # flush emitted BEFORE the next superblock's tile stream: its
            # ACT/DVE/PE ops then fill the gather-completion gap that gates
            # the next scales (in-order engines would otherwise idle).
            issue_gathers(1)
            pending = None
            for sbi in range(NSB):
                if sbi + 2 < NSB:
                    issue_gathers(sbi + 2)
                if pending is not None:
                    flush_of(*pending)
                banks = tiles_of(sbi)
                pending = (sbi, *banks)
            flush_of(*pending)

        body()


# --------------------------------------------------------------- entry point

def _build_program(geom, cfg):
    import concourse.bacc as bacc
    import concourse.mybir as mybir
    import concourse.tile as tile

    f32 = mybir.dt.float32
    f16 = mybir.dt.float16
    i16 = mybir.dt.int16

    nc = bacc.Bacc("TRN2", target_bir_lowering=False, debug=False,
                   enable_asserts=False, num_devices=NCORES,
                   num_swdge_queues=4)

    def inp(name, shape, dt):
        return nc.dram_tensor(name, shape, dt, kind="ExternalInput").ap()

    gidx = inp("gidx", [P, geom["GIDXC"]], i16)
    gdegS = inp("gdegS", [P, geom["TILES"]], i16)
    gdegD = inp("gdegD", [P, geom["TILES"]], i16)
    featsT = inp("featsT", [P, NLOC_PAD], f16)
    masks = inp("masks", [P, geom["NMASK"] * 1024], f16)
    wmT = inp("wmT", [P, D], f16)
    wsT = inp("wsT", [P, D], f16)
    bm = inp("bm", [1, D], f32)
    bs = inp("bs", [1, D], f32)
    tabs = [inp(f"tab{ph}", [geom["TABR"], D], f16) for ph in range(NPHASE)]
    out = nc.dram_tensor("out", [NLOC_PAD, D], f32, kind="ExternalOutput").ap()

    with tile.TileContext(nc) as tc:
        device_kernel(
            tc, [out],
            [gidx, gdegS, gdegD, featsT, masks, wmT, wsT, bm, bs, tabs],
            geom, cfg)
    nc.compile()
    return nc


LAST_EXEC_NS = None


def kernel(feats, src, dst, linear_skip_weight, linear_skip_bias,
           linear_msg_weight, linear_msg_bias):
    global LAST_EXEC_NS
    import os

    from concourse.bass_utils import run_bass_kernel_spmd

    feats = np.asarray(feats)
    in_maps, geom, order = _prep(
        feats, src, dst, linear_msg_weight, linear_msg_bias,
        linear_skip_weight, linear_skip_bias)
    nc = _build_program(geom, cfg={})
    trace = bool(int(os.environ.get("GCN_TRACE", "0")))
    res = run_bass_kernel_spmd(nc, in_maps, core_ids=list(range(NCORES)),
                               trace=trace)
    LAST_EXEC_NS = res.exec_time_ns
    if res.instructions_and_trace is not None:
        print("trace:", res.instructions_and_trace[1])
    out = np.empty((N, D), np.float32)
    rr = np.arange(NLOC)
    for k in range(NCORES):
        out[order[rr * NCORES + k]] = res.results[k]["out"][:NLOC]
    return out


# revision 12
# speedup vs baseline: 2.2823x; 2.2823x over previous
"""GCN layer (symmetric-normalized message passing + skip) on 8 Trainium2
NeuronCores via Bass/Tile.

    deg = bincount(src); dis = (deg>0) * rsqrt(max(deg,1))
    out = segsum_dst( dis_src*dis_dst * feats[src] ) @ Wm.T + bm
          + feats @ Ws.T + bs

v3 design (vs v2's dynamic one-hot): STATIC scatter masks. Nodes are sorted
globally by in-degree and dealt round-robin to the 8 cores, so every core
sees the same degree profile. Each 512-dst superblock assigns every dst a
fixed, even K = max in-degree in the block rounded up; edge j of dst d sits
at slot d*K+j. The scatter matrix for each 128-slot tile is then a STATIC
0/1 band that depends only on (K, 128t mod K) -- 19 distinct [128,1024] f16
mask buffers loaded once, with the matmul rhs a 512-wide column slice.
This removes the per-tile DVE one-hot build (v2's #1 compute load) and the
per-edge ACT scaling entirely: the only per-tile vector work is one
[128,128] tensor_scalar multiplying the gathered messages by the per-slot
weight w = dis[src]*dis[dst] (computed on device from integer degrees; pad
slots have w=0, which also nullifies garbage gather rows).

Gather: superblock sb uses a per-(core, sb%4) compacted source table
(<32768 rows, so one int16-indexed dma_gather covers a whole slot range);
each superblock's rows split into 4 chunks, one per SWDGE queue, which run
concurrently on separate Q7 core pairs. Trailing pad slots per chunk get
idx=-1, which the ucode trims for free.

Host prep supplies integer partitioning metadata only (permutations, table
row ids, per-slot degrees); all float math runs on device.
"""

import numpy as np

P = 128
D = 128
NCORES = 8
N = 100000
E = 640000
SBW = 512                   # superblock width (dst nodes)
NSB = 25                    # superblocks per core (25*512 = 12800 >= 12500)
NLOC = N // NCORES          # 12500
NLOC_PAD = NSB * SBW        # 12800
NPHASE = 4                  # sb % 4 -> which compacted source table
MAXIDX = 32768              # int16 gather index limit


# ---------------------------------------------------------------- host prep

def _prep(feats, src, dst, wm, bm, ws, bs):
    n, d = feats.shape
    assert n == N and d == D
    src = np.asarray(src).astype(np.int64)
    dst = np.asarray(dst).astype(np.int64)
    feats = np.asarray(feats, dtype=np.float32)

    deg = np.bincount(src, minlength=n)            # out-degree -> dis
    keep = deg[dst] > 0                            # dis[dst]=0 edges are 0
    src_k, dst_k = src[keep], dst[keep]
    deg_in = np.bincount(dst_k, minlength=n)

    order = np.argsort(-deg_in, kind="stable")     # rank -> node
    rank_of = np.empty(n, np.int64)
    rank_of[order] = np.arange(n)
    owner = rank_of % NCORES
    lrank = rank_of // NCORES

    Ks = []
    for sb in range(NSB):
        band = deg_in[order[sb * SBW * NCORES:(sb + 1) * SBW * NCORES]]
        K = max(2, int(band.max()))
        K += K % 2                                 # even K: few mask bufs
        Ks.append(K)
    Ks = np.array(Ks)
    tiles_sb = (SBW * Ks // P).astype(np.int64)    # = 4*K
    slot_base = np.concatenate([[0], np.cumsum(SBW * Ks)]).astype(np.int64)
    TOT = int(slot_base[-1])
    TILES = TOT // P

    # static mask buffers, one per distinct (K, c); band at col 512+(c+p)//K
    mask_key = {}
    bufs = []
    for sb in range(NSB):
        K = int(Ks[sb])
        for t in range(int(tiles_sb[sb])):
            c = (P * t) % K
            if (K, c) not in mask_key:
                buf = np.zeros((P, 1024), np.float16)
                pp = np.arange(P)
                buf[pp, 512 + (c + pp) // K] = 1.0
                mask_key[(K, c)] = len(bufs)
                bufs.append(buf)
    NMASK = len(bufs)
    masks = np.zeros((P, NMASK * 1024), np.float16)
    for mi, buf in enumerate(bufs):
        masks[:, mi * 1024:(mi + 1) * 1024] = buf

    # per-tile mask slice (buffer id, column start) -- same for all cores
    tile_mask = []                                 # [(mi, col_start)] * TILES
    for sb in range(NSB):
        K = int(Ks[sb])
        for t in range(int(tiles_sb[sb])):
            c = (P * t) % K
            d0 = (P * t) // K
            mi = mask_key[(K, c)]
            tile_mask.append((mi, mi * 1024 + 512 - d0))

    # gather chunking: ops of <=TPO tiles (TPO*128 rows; TPO*8+2 descs/engine
    # must fit the per-queue SWDGE ring), round-robin across the 4 queues
    import os as _os
    TPO = int(_os.environ.get("GCN_TPO", "7"))
    chunks = []                                    # (sb, tile_lo, tile_hi, q)
    qctr = 0
    for sb in range(NSB):
        tsb = int(tiles_sb[sb])
        t0 = int(slot_base[sb]) // P
        for t in range(0, tsb, TPO):
            lo, hi = t0 + t, t0 + min(tsb, t + TPO)
            chunks.append((sb, lo, hi, qctr % 4))
            qctr += 1
    # gidx column base per chunk: rows/16 columns each, laid sequentially
    chunk_cols = []
    cb = 0
    for (_, lo, hi, _) in chunks:
        nc_ = (hi - lo) * P // 16
        chunk_cols.append((cb, nc_))
        cb += nc_
    GIDXC = cb                                     # == TOT // 16

    # per-core tables and index/degree metadata
    per_core = []
    for k in range(NCORES):
        m = owner[dst_k] == k
        s_e, d_e = src_k[m], dst_k[m]
        lr = lrank[d_e]
        o2 = np.argsort(lr, kind="stable")
        s_e, d_e, lr = s_e[o2], d_e[o2], lr[o2]
        sb_e, dloc = lr // SBW, lr % SBW
        j = np.arange(len(lr)) - np.searchsorted(lr, lr)
        slot = slot_base[sb_e] + dloc * Ks[sb_e] + j

        phase_e = sb_e % NPHASE
        tabs = []
        gidx_flat = np.zeros(TOT, np.int64)
        for ph in range(NPHASE):
            mm2 = phase_e == ph
            uniq = np.unique(s_e[mm2])
            assert len(uniq) < MAXIDX, len(uniq)
            r = np.zeros(n, np.int64)
            r[uniq] = np.arange(len(uniq))
            tabs.append(feats[uniq].astype(np.float16))
            gidx_flat[slot[mm2]] = r[s_e[mm2]]

        filled = np.zeros(TOT, bool)
        filled[slot] = True
        degS = np.zeros(TOT, np.int16)
        degD = np.zeros(TOT, np.int16)
        degS[slot] = deg[s_e].astype(np.int16)
        degD[slot] = deg[d_e].astype(np.int16)

        # trailing pad slots per chunk -> idx -1 (ucode trims them)
        for (sb, lo, hi, _), (cb0, ncol) in zip(chunks, chunk_cols):
            s0, s1 = lo * P, hi * P
            f = filled[s0:s1]
            nz = np.flatnonzero(f)
            tail = (nz[-1] + 1) if len(nz) else 0
            gidx_flat[s0 + tail:s1] = -1

        # wrap-16 + replicate-8 packing per chunk
        gidx = np.zeros((P, GIDXC), np.int16)
        for (sb, lo, hi, _), (cb0, ncol) in zip(chunks, chunk_cols):
            fl = gidx_flat[lo * P:hi * P].astype(np.int16)
            a = fl.reshape(ncol, 16).T              # [16, ncol]
            gidx[:, cb0:cb0 + ncol] = np.tile(a, (8, 1))

        gdegS = np.ascontiguousarray(degS.reshape(TILES, P).T)
        gdegD = np.ascontiguousarray(degD.reshape(TILES, P).T)

        ft = np.zeros((P, NLOC_PAD), np.float16)
        rr = np.arange(NLOC)
        ft[:, :NLOC] = feats[order[rr * NCORES + k]].T.astype(np.float16)

        per_core.append((tabs, gidx, gdegS, gdegD, ft))

    # uniform table shape across cores/phases
    TABR = max(t.shape[0] for (tabs, *_r) in per_core for t in tabs)
    TABR = (TABR + 127) // 128 * 128

    wmT = np.ascontiguousarray(np.asarray(wm, np.float32).T).astype(np.float16)
    wsT = np.ascontiguousarray(np.asarray(ws, np.float32).T).astype(np.float16)
    bm = np.asarray(bm, np.float32).reshape(1, D)
    bs = np.asarray(bs, np.float32).reshape(1, D)

    in_maps = []
    for k in range(NCORES):
        tabs, gidx, gdegS, gdegD, ft = per_core[k]
        im = {"gidx": gidx, "gdegS": gdegS, "gdegD": gdegD,
              "featsT": ft, "masks": masks,
              "wmT": wmT, "wsT": wsT, "bm": bm, "bs": bs}
        for ph in range(NPHASE):
            tt = np.zeros((TABR, D), np.float16)
            tt[:tabs[ph].shape[0]] = tabs[ph]
            im[f"tab{ph}"] = tt
        in_maps.append(im)

    geom = {
        "Ks": Ks.tolist(), "tiles_sb": tiles_sb.tolist(),
        "slot_base": slot_base.tolist(), "TOT": TOT, "TILES": TILES,
        "NMASK": NMASK, "tile_mask": tile_mask, "chunks": chunks,
        "chunk_cols": chunk_cols, "GIDXC": GIDXC, "TABR": TABR,
    }
    return in_maps, geom, order


# ------------------------------------------------------------- device kernel

def device_kernel(tc, outs, ins, geom, cfg):
    import concourse.mybir as mybir

    nc = tc.nc
    f32 = mybir.dt.float32
    f16 = mybir.dt.float16
    i16 = mybir.dt.int16
    Op = mybir.AluOpType
    Act = mybir.ActivationFunctionType

    (out_d,) = outs
    (gidx_d, gdegS_d, gdegD_d, featsT_d, masks_d,
     wmT_d, wsT_d, bm_d, bs_d, tab_ds) = ins

    TILES = geom["TILES"]
    NMASK = geom["NMASK"]
    GIDXC = geom["GIDXC"]
    tiles_sb = geom["tiles_sb"]
    slot_base = geom["slot_base"]
    tile_mask = geom["tile_mask"]
    chunks = geom["chunks"]
    chunk_cols = geom["chunk_cols"]

    NBUF = 3
    bufmax = [max(tiles_sb[sb] for sb in range(i, NSB, NBUF))
              for i in range(NBUF)]

    abl = cfg.get("ABL", ())

    with (
        tc.tile_pool(name="sbuf", bufs=1) as sb,
        tc.tile_pool(name="srst", bufs=2) as srst,
        tc.tile_pool(name="sstg", bufs=4) as sstg,
        tc.tile_pool(name="psag", bufs=2, space="PSUM") as psag,
        tc.tile_pool(name="pslin", bufs=4, space="PSUM") as pslin,
    ):
        # ---------------- setup ----------------
        # gidx first: sb0 gathers depend only on it (+ tables already in
        # HBM), so they issue under the remaining input loads + w compute.
        gidx = sb.tile([P, GIDXC], i16)
        nc.sync.dma_start(out=gidx[:], in_=gidx_d[:])

        msgs0 = sb.tile([P, bufmax[0] * P], f16, tag="msgs0")
        msgs1 = sb.tile([P, bufmax[1] * P], f16, tag="msgs1")
        msgs2 = sb.tile([P, bufmax[2] * P], f16, tag="msgs2")
        msgs_bufs = [msgs0, msgs1, msgs2]

        sb_chunks = {}
        for (sbi, lo, hi, q), (cb0, ncol) in zip(chunks, chunk_cols):
            sb_chunks.setdefault(sbi, []).append((lo, hi, q, cb0, ncol))

        def issue_gathers(sbi):
            msgs = msgs_bufs[sbi % NBUF]
            if "gather" in abl:
                return
            t0 = slot_base[sbi] // P
            for (lo, hi, q, cb0, ncol) in sb_chunks[sbi]:
                nrow = (hi - lo) * P
                nc.gpsimd.dma_gather(
                    msgs[:, (lo - t0) * P:(hi - t0) * P]
                    .rearrange("p (t e) -> p t e", e=D),
                    tab_ds[sbi % NPHASE][:, :],
                    gidx[:, cb0:cb0 + ncol],
                    nrow, nrow, D, queue_num=q)

        issue_gathers(0)

        maskt = sb.tile([P, NMASK * 1024], f16)
        nc.sync.dma_start(out=maskt[:], in_=masks_d[:])
        wmT = sb.tile([P, D], f16)
        nc.sync.dma_start(out=wmT[:], in_=wmT_d[:])
        wsT = sb.tile([P, D], f16)
        nc.sync.dma_start(out=wsT[:], in_=wsT_d[:])
        featsT = sb.tile([P, NLOC_PAD], f16)
        nc.sync.dma_start(out=featsT[:], in_=featsT_d[:])

        # bias16 = (bm + bs) as f16 row
        bmt = sb.tile([1, D], f32)
        nc.sync.dma_start(out=bmt[:], in_=bm_d[:])
        bst = sb.tile([1, D], f32)
        nc.sync.dma_start(out=bst[:], in_=bs_d[:])
        nc.vector.tensor_tensor(out=bmt[:], in0=bmt[:], in1=bst[:], op=Op.add)
        bias16 = sb.tile([1, D], f16)
        nc.vector.tensor_copy(out=bias16[:], in_=bmt[:])
        ones1 = sb.tile([1, P], f16)
        nc.vector.memset(ones1[:], 1.0)

        # wE[p, t] = dis(degS)*dis(degD), dis(x) = (x>0)*rsqrt(max(x,1))
        def dis_of(deg_d):
            di = sb.tile([P, TILES], i16, tag=f"digi{id(deg_d)}")
            nc.sync.dma_start(out=di[:], in_=deg_d[:])
            df = sb.tile([P, TILES], f32, tag="digf")
            nc.vector.tensor_copy(out=df[:], in_=di[:])
            msk = sb.tile([P, TILES], f32, tag="dmsk")
            nc.vector.tensor_scalar(out=msk[:], in0=df[:], scalar1=0.0,
                                    scalar2=None, op0=Op.is_gt)
            nc.vector.tensor_scalar(out=df[:], in0=df[:], scalar1=1.0,
                                    scalar2=None, op0=Op.max)
            rc = sb.tile([P, TILES], f32, tag="drc")
            nc.vector.reciprocal(out=rc[:], in_=df[:])
            rt = sb.tile([P, TILES], f32, tag="drt")
            nc.scalar.activation(out=rt[:], in_=rc[:], func=Act.Sqrt)
            w = sb.tile([P, TILES], f32, tag=f"dw{id(deg_d)}")
            nc.vector.tensor_tensor(out=w[:], in0=rt[:], in1=msk[:],
                                    op=Op.mult)
            return w

        wS = dis_of(gdegS_d)
        wD = dis_of(gdegD_d)
        wE = sb.tile([P, TILES], f32)
        nc.vector.tensor_tensor(out=wE[:], in0=wS[:], in1=wD[:], op=Op.mult)

        # ---------------- main loop ----------------
        def tiles_of(sbi):
            msgs = msgs_bufs[sbi % NBUF]
            t0 = slot_base[sbi] // P
            tsb = tiles_sb[sbi]
            bankA = psag.tile([P, SBW], f32, tag="aggA", space="PSUM")
            bankB = psag.tile([P, SBW], f32, tag="aggB", space="PSUM")
            for t in range(tsb):
                T0 = t0 + t
                if "scale" not in abl:
                    nc.vector.tensor_scalar(
                        out=msgs[:, t * P:(t + 1) * P],
                        in0=msgs[:, t * P:(t + 1) * P],
                        scalar1=wE[:, T0:T0 + 1],
                        scalar2=None, op0=Op.mult)
                if "aggmm" in abl:
                    continue
                mi, cs = tile_mask[T0]
                bank = bankA if t % 2 == 0 else bankB
                nc.tensor.matmul(
                    out=bank[:], lhsT=msgs[:, t * P:(t + 1) * P],
                    rhs=maskt[:, cs:cs + SBW],
                    start=(t < 2), stop=(t >= tsb - 2))
            return bankA, bankB

        def flush_of(sbi, bankA, bankB):
            if "aggmm" in abl or "flush" in abl:
                return
            rstT = srst.tile([P, SBW], f16, tag="rstT")
            nc.scalar.copy(out=rstT[:], in_=bankA[:])
            nc.vector.tensor_tensor(out=rstT[:], in0=bankB[:],
                                    in1=rstT[:], op=Op.add)
            for b in range(4):
                pmk = pslin.tile([P, D], f32, tag="pmk", space="PSUM")
                nc.tensor.matmul(out=pmk[:],
                                 lhsT=rstT[:, b * P:(b + 1) * P],
                                 rhs=wmT[:], start=True, stop=False)
                nc.tensor.matmul(out=pmk[:],
                                 lhsT=featsT[:, (sbi * 4 + b) * P:
                                             (sbi * 4 + b + 1) * P],
                                 rhs=wsT[:], start=False, stop=False)
                nc.tensor.matmul(out=pmk[:], lhsT=ones1[:], rhs=bias16[:],
                                 start=False, stop=True)
                stage = sstg.tile([P, D], f32, tag="stage")
                nc.scalar.copy(out=stage[:], in_=pmk[:])
                nc.sync.dma_start(
                    out=out_d[(sbi * 4 + b) * P:(sbi * 4 + b + 1) * P, :],
                    in_=stage[:])

        def body():
            # flush emitted BEFORE the next superblock's tile stream: its
            # ACT/DVE/PE ops then fill the gather-completion gap that gates
            # the next scales (in-order engines would otherwise idle).
            issue_gathers(1)
            pending = None
            for sbi in range(NSB):
                if sbi + 2 < NSB:
                    issue_gathers(sbi + 2)
                if pending is not None:
                    flush_of(*pending)
                banks = tiles_of(sbi)
                pending = (sbi, *banks)
            flush_of(*pending)

        body()


# --------------------------------------------------------------- entry point

def _build_program(geom, cfg):
    import concourse.bacc as bacc
    import concourse.mybir as mybir
    import concourse.tile as tile

    f32 = mybir.dt.float32
    f16 = mybir.dt.float16
    i16 = mybir.dt.int16

    nc = bacc.Bacc("TRN2", target_bir_lowering=False, debug=False,
                   enable_asserts=False, num_devices=NCORES,
                   num_swdge_queues=4)

    def inp(name, shape, dt):
        return nc.dram_tensor(name, shape, dt, kind="ExternalInput").ap()

    gidx = inp("gidx", [P, geom["GIDXC"]], i16)
    gdegS = inp("gdegS", [P, geom["TILES"]], i16)
    gdegD = inp("gdegD", [P, geom["TILES"]], i16)
    featsT = inp("featsT", [P, NLOC_PAD], f16)
    masks = inp("masks", [P, geom["NMASK"] * 1024], f16)
    wmT = inp("wmT", [P, D], f16)
    wsT = inp("wsT", [P, D], f16)
    bm = inp("bm", [1, D], f32)
    bs = inp("bs", [1, D], f32)
    tabs = [inp(f"tab{ph}", [geom["TABR"], D], f16) for ph in range(NPHASE)]
    out = nc.dram_tensor("out", [NLOC_PAD, D], f32, kind="ExternalOutput").ap()

    with tile.TileContext(nc) as tc:
        device_kernel(
            tc, [out],
            [gidx, gdegS, gdegD, featsT, masks, wmT, wsT, bm, bs, tabs],
            geom, cfg)
    nc.compile()
    return nc


LAST_EXEC_NS = None


def kernel(feats, src, dst, linear_skip_weight, linear_skip_bias,
           linear_msg_weight, linear_msg_bias):
    global LAST_EXEC_NS
    import os

    from concourse.bass_utils import run_bass_kernel_spmd

    feats = np.asarray(feats)
    in_maps, geom, order = _prep(
        feats, src, dst, linear_msg_weight, linear_msg_bias,
        linear_skip_weight, linear_skip_bias)
    abl = tuple(a for a in os.environ.get("GCN_ABL", "").split(",") if a)
    nc = _build_program(geom, cfg={"ABL": abl} if abl else {})
    trace = bool(int(os.environ.get("GCN_TRACE", "0")))
    res = run_bass_kernel_spmd(nc, in_maps, core_ids=list(range(NCORES)),
                               trace=trace)
    LAST_EXEC_NS = res.exec_time_ns
    if res.instructions_and_trace is not None:
        print("trace:", res.instructions_and_trace[1])
    out = np.empty((N, D), np.float32)
    rr = np.arange(NLOC)
    for k in range(NCORES):
        out[order[rr * NCORES + k]] = res.results[k]["out"][:NLOC]
    return out
